# revision 29
# baseline (speedup 1.0000x reference)
"""Trainium2 Bass kernel for nn_ContrastLoss (supervised-contrastive loss).

Reference computation (B=1024, D=128, C=100, K=32768, N=B+K=33792):
    l   = concat(labels, queue_label.T)          # [N, C]
    w   = labels @ l.T                           # [B, N] shared-class counts
    sim = query @ concat(keys, queue.T).T / T    # [B, N]
    logits = sim - rowmax(sim)
    denom  = sum(exp(logits) * logits_mask, 1)   # logits_mask zeros keys-diag
    loss = -(T/BT) * sqrt(w/max(w)) * (logits - log(denom))

Key restructurings:
  * max(w) == max_i rowsum(labels_i) exactly (binary labels; any cross dot
    is |intersection| <= min row count).  Computed on host (tiny prep).
  * Constant softmax stabilizer m=1.0 is numerically safe (sim in [-1,1]
    for L2-normalized inputs), so no rowmax pass is needed.
  * loss = s * (lnD + L')  with  L' = (m-raw)/T  (an affine of the raw GEMM
    result -- an ACT Copy, no transcendental), lnD = Ln(denom) (ONE tiny
    [128,1] Ln), s = sqrt(w*(T/BT)^2/wmax) from an exact fp8 GEMM.
    This removes the serial full-size Ln pass entirely: the Exp pass (for
    denominators) and the Copy pass both run in phase 1 where ACT has
    slack under the Tensor-bound fp32 GEMM; phase 2 is Sqrt + one DVE
    scalar_tensor_tensor (lnD rides in as the per-partition scalar).

Sharding: pure data-parallel over the 1024 query rows -- core c owns rows
[c*128, (c+1)*128) and computes the FULL 33792 similarity columns for them,
including its own row-denominators.  NO cross-core collective (the
baseline's tiny AllGather serialized all 8 cores on launch skew).

Per-core phases (CH=1024-column chunks, 33 of them; phase 1 is
DMA/Tensor co-bound, phase 2 DVE-bound):
  phase 1: fp32 sim GEMM chunk -> PSUM (rhs streamed over both HWDGE
           queues in 512 KiB chunks);
           DVE tensor_scalar: L'[:, chunk] = raw*(-1/T) + m/T -> SBUF
           resident [128, N] f32;
           ACT Exp: exp((raw-m)/T) -> throwaway bf16 ring + accum rowsum.
  bridge:  denom = sum(accs) - e_diag (host-computed corr input);
           lnD = Ln(denom)  ([128,1]; same ACT table set as Exp).
  phase 2: fp8 w GEMM chunk -> ACT Sqrt(scale=k2) -> s;
           DVE stt: out = (L' + lnD) * s -> bf16 -> DMA.

ACT table sets: Exp and Ln live in natural_log_exp_and_others; Sqrt in
sqrt_and_others -> the table set swaps only when the scheduler
interleaves phases (hidden under phase-1 ACT idle).

Output returned as bf16 (rel err 2^-9 ~ 0.4% of each element's own
magnitude, far inside the 2e-2 gate), halving output DMA traffic.
"""

import numpy as np
import ml_dtypes

import concourse.bass as bass
import concourse.mybir as mybir
import concourse.tile as tile
from concourse import bacc
from concourse.bass_utils import run_bass_kernel_spmd

F32 = mybir.dt.float32
BF16 = mybir.dt.bfloat16
FP8 = mybir.dt.float8e4
ALU = mybir.AluOpType
ACTF = mybir.ActivationFunctionType

B, D, C, KQ = 1024, 128, 100, 32768
N = B + KQ             # 33792 similarity columns
NCORES = 8
RPC = B // NCORES      # 128 rows per core
CH = 1024              # column chunk (2 x 512-col matmuls per PSUM tile)
NCH = N // CH          # 33
STAB = 1.0             # softmax stabilizer m (raw sim values are in [-1, 1])


def _build_nc(Tf: float, BTf: float, wmax: float):
    nc = bacc.Bacc("TRN2", target_bir_lowering=False, debug=False,
                   num_devices=NCORES)

    # chunk-major DRAM layouts: every streamed transfer is one fully
    # contiguous block (sequential HBM access instead of 128 strided rows).
    qT_d = nc.dram_tensor("qT", [D, RPC], F32, kind="ExternalInput")
    rhs_sim_d = nc.dram_tensor("rhs_sim", [NCH, D, CH], F32,
                               kind="ExternalInput")
    labT_d = nc.dram_tensor("labT", [C, RPC], FP8, kind="ExternalInput")
    rhs_w_d = nc.dram_tensor("rhs_w", [4, C, N // 4], FP8,
                             kind="ExternalInput")
    corr_d = nc.dram_tensor("corr", [RPC, 1], F32, kind="ExternalInput")
    out_d = nc.dram_tensor("out", [NCH, RPC, CH], BF16,
                           kind="ExternalOutput")

    invT = 1.0 / Tf
    sq_scale = (Tf / BTf) ** 2 / wmax

    with tile.TileContext(nc) as tc:
        with (
            tc.tile_pool(name="const", bufs=1) as const,
            tc.tile_pool(name="lbig", bufs=1) as lbig,
            tc.tile_pool(name="rhs", bufs=4) as rhsp,
            tc.tile_pool(name="work", bufs=2) as work,
            tc.tile_pool(name="outp", bufs=3) as outp,
            tc.tile_pool(name="psum", bufs=2, space="PSUM") as psum,
        ):
            # ---- stationary operand first (tiny, unblocks first matmul) ---
            qT = const.tile([D, RPC], F32)
            nc.sync.dma_start(out=qT[:], in_=qT_d[:])

            ebias = const.tile([RPC, 1], F32)
            nc.vector.memset(ebias, -STAB / Tf)
            zbias = const.tile([RPC, 1], F32)
            nc.vector.memset(zbias, 0.0)

            labT = const.tile([C, RPC], FP8)
            rhs_w = const.tile([C, N], FP8)
            corr = const.tile([RPC, 1], F32)

            acc = const.tile([RPC, NCH], F32)
            lp = lbig.tile([RPC, N], F32)  # L' = (m-raw)/T, SBUF-resident

            # ---- phase 1: fp32 sim matmul -> L' store; Exp+accum ----------
            # Deferred const loads ride the SWDGE (gpsimd) queue so the big
            # rhs_w transfer cannot head-of-line-block the HWDGE rhs stream;
            # rhs_w is also split in four to spread its bandwidth use.
            WQ = N // 4
            for i in range(NCH):
                base = i * CH
                r = rhsp.tile([D, CH], F32, tag="rhs")
                # alternate the two HWDGE rings (SP + ACT) for issue overlap
                dma_eng = nc.sync if i % 2 == 0 else nc.scalar
                dma_eng.dma_start(out=r[:], in_=rhs_sim_d[i])
                if i == 4:
                    nc.gpsimd.dma_start(out=labT[:], in_=labT_d[:])
                    nc.gpsimd.dma_start(out=corr[:], in_=corr_d[:])
                if i in (26, 28, 30, 32):
                    j = (i - 26) // 2
                    nc.gpsimd.dma_start(
                        out=rhs_w[:, j * WQ:(j + 1) * WQ],
                        in_=rhs_w_d[j])
                ps = psum.tile([RPC, CH], F32, tag="psE")
                nc.tensor.matmul(ps[:, 0:512], qT[:], r[:, 0:512],
                                 start=True, stop=True)
                nc.tensor.matmul(ps[:, 512:CH], qT[:], r[:, 512:CH],
                                 start=True, stop=True)
                # L' = raw*(-1/T) + m/T on DVE (PSUM-source tensor_scalar);
                # Exp on ACT reads the same PSUM tile concurrently.
                nc.vector.tensor_scalar(
                    out=lp[:, base:base + CH], in0=ps[:],
                    scalar1=-invT, scalar2=STAB / Tf,
                    op0=ALU.mult, op1=ALU.add)
                scr = work.tile([RPC, CH], BF16, tag="escr")
                nc.scalar.activation(scr[:], ps[:], ACTF.Exp,
                                     bias=ebias[:], scale=invT,
                                     accum_out=acc[:, i:i + 1])

            # ---- denominators (self-diag removed) -> lnD ------------------
            dn = const.tile([RPC, 1], F32)
            nc.vector.tensor_reduce(dn[:], acc[:], axis=mybir.AxisListType.X,
                                    op=ALU.add)
            dn2 = const.tile([RPC, 1], F32)
            nc.vector.tensor_sub(dn2[:], dn[:], corr[:])
            lnD = const.tile([RPC, 1], F32)
            nc.scalar.activation(lnD[:], dn2[:], ACTF.Ln,
                                 bias=zbias[:], scale=1.0)

            # ---- phase 2: w matmul -> s = Sqrt(w*k2); out = (L'+lnD)*s ----
            for i in range(NCH):
                base = i * CH
                psw = psum.tile([RPC, CH], F32, tag="psB")
                nc.tensor.matmul(psw[:, 0:512], labT[:],
                                 rhs_w[:, base:base + 512],
                                 start=True, stop=True)
                nc.tensor.matmul(psw[:, 512:CH], labT[:],
                                 rhs_w[:, base + 512:base + CH],
                                 start=True, stop=True)
                s = work.tile([RPC, CH], F32, tag="s")
                nc.scalar.activation(s[:], psw[:], ACTF.Sqrt,
                                     bias=zbias[:], scale=sq_scale)
                o = outp.tile([RPC, CH], BF16, tag="o")
                nc.vector.scalar_tensor_tensor(
                    o[:], lp[:, base:base + CH], lnD[:], s[:],
                    op0=ALU.add, op1=ALU.mult)
                nc.sync.dma_start(out=out_d[i], in_=o[:])

    nc.compile()
    return nc


def _host_prep(query, keys, labels, queue, queue_label, Tf):
    fp8 = ml_dtypes.float8_e4m3
    query = np.asarray(query, np.float32)
    keys = np.asarray(keys, np.float32)
    labels = np.asarray(labels, np.float32)
    queue = np.asarray(queue, np.float32)
    queue_label = np.asarray(queue_label, np.float32)

    rhs_sim = np.concatenate([keys.T, queue], axis=1).astype(np.float32)
    rhs_sim = np.ascontiguousarray(                    # [NCH, D, CH]
        rhs_sim.reshape(D, NCH, CH).transpose(1, 0, 2))
    rhs_w = np.concatenate([labels.T, queue_label], axis=1).astype(fp8)
    rhs_w = np.ascontiguousarray(                      # [4, C, N//4]
        rhs_w.reshape(C, 4, N // 4).transpose(1, 0, 2))
    qT_full = np.ascontiguousarray(query.T, dtype=np.float32)      # [D, B]
    labT_full = np.ascontiguousarray(labels.T.astype(fp8))         # [C, B]
    wmax = float(labels.sum(axis=1).max())
    # e at the self (keys-block) diagonal: removed from each row's denom.
    diag_sim = np.einsum("bd,bd->b", query, keys, dtype=np.float64)
    corr_full = np.exp((diag_sim - STAB) / Tf).astype(np.float32)  # [B]

    in_maps = []
    for c in range(NCORES):
        rows = slice(c * RPC, (c + 1) * RPC)
        in_maps.append({
            "qT": np.ascontiguousarray(qT_full[:, rows]),
            "rhs_sim": rhs_sim,
            "labT": np.ascontiguousarray(labT_full[:, rows]),
            "rhs_w": rhs_w,
            "corr": np.ascontiguousarray(corr_full[rows]).reshape(RPC, 1),
        })
    return in_maps, wmax


def _gather_output(results):
    rows = []
    for c in range(NCORES):
        r = np.asarray(results[c]["out"], dtype=np.float32)  # [NCH, RPC, CH]
        rows.append(r.transpose(1, 0, 2).reshape(RPC, N))
    return np.concatenate(rows, axis=0)


def kernel(query, keys, labels, queue, queue_label, K, T, BT, **_unused):
    Tf = float(np.asarray(T))
    BTf = float(np.asarray(BT))
    in_maps, wmax = _host_prep(query, keys, labels, queue, queue_label, Tf)
    nc = _build_nc(Tf, BTf, wmax)
    res = run_bass_kernel_spmd(nc, in_maps, list(range(NCORES)))
    return _gather_output(res.results)


# Re-usable entry for test.py: returns (output, BassKernelResults) so the
# harness there can pull exec_time_ns / profile out of a traced run.
def kernel_traced(query, keys, labels, queue, queue_label, K, T, BT,
                  trace=False, **run_kwargs):
    Tf = float(np.asarray(T))
    BTf = float(np.asarray(BT))
    in_maps, wmax = _host_prep(query, keys, labels, queue, queue_label, Tf)
    nc = _build_nc(Tf, BTf, wmax)
    res = run_bass_kernel_spmd(nc, in_maps, list(range(NCORES)),
                               trace=trace, **run_kwargs)
    return _gather_output(res.results), res
